# revision 36
# baseline (speedup 1.0000x reference)
"""Trainium2 Bass kernel for nn_MultiHeadAttention_89318139888179.

Problem: B=4, S=2048, D=1024, H=16 heads (hd=64) fp32 multi-head attention
with (quirky) RoPE, y = softmax((rot(q) @ rot(k)^T)/8) v, projections are
x @ W^T + b with W [e,d].

Sharding: 8 cores = 4 batches x 2 head-halves. Each core computes Q/K/V for
its 8 heads over the batch's full 2048 queries, attention per-head local,
and a row-sharded out-projection partial (contraction over its 512 ctx
dims). The host sums each batch pair's partials during unshard (allowed:
kernel() takes full inputs / returns the full output; no collectives).
This halves the K/V projection work vs (batch x query-half) sharding,
which computed every batch's K/V twice.

Per-core layout (all matmul contractions on SBUF partitions):
 - Projections run in fp8(e4m3) DoubleRow perf mode (2 contraction tiles
   per pass at 0.5 cycles/row) with residual compensation:
   y = Wh.xh + Wh.xl + Wl.xh where t = t_hi + t_lo is an fp8 hi/lo split
   (hi = fp8(t), lo = fp8(t - hi)); the dropped Wl.xl term is O(eps^2).
   W rows are host-scaled by 32 so sigma~1 (raw sigma=1/32 sits in e4m3's
   subnormal flush zone); the PSUM->SBUF copy rescales by 1/32. Measured
   per-projection error ~0.13%, slightly better than bf16. 25% fewer
   PE column-passes than bf16 (6 vs 8 passes per 512 contraction).
 - Host interleaves Wq/Wk output rows so RoPE rotation pairs sit on
   adjacent (even,odd) partitions: rotation = DVE stream_shuffle + 2 mul
   + add against sin/cos tables.
 - scoresT[k, q] = K^T.T @ Q^T per head (bf16, K=64), exp on the scalar
   engine (scale=1/8 fused, no max subtraction: |logits| < ~6), PV as
   ctx^T = V_aug.T @ expT with a ones column so PSUM row 64 of the ctx
   accumulator is the softmax denominator (bf16: the M=65 row is free,
   matmul cost depends only on the moving dim).
 - 1/Z on the DVE (reciprocal), broadcast across partitions by the idle
   GPSIMD, normalize on DVE -- keeps the Activation engine exp-only
   (it is the second-busiest engine at ~0.9x the PE).
 - Normalized ctx^T head-pairs pack into a resident [128, 4, 2048] tile,
   are hi/lo split (x32 scale, one tile per group pair -- avoids coarse-AP
   false deps) for the fp8 out-projection; output lands [s, e] fp32 (psum
   copy rescales by 2^-10) and DMAs straight out.
 - Schedule: the attention key loop is software-pipelined (PV one kt2
   behind scores/exp; each block's last PV + normalize carried into the
   next block) and all projection / V / out-proj work is spread as PE
   filler inside the key loops, so the in-order PE stream never waits on
   the exp pipeline. TimelineSim: ~339.1 us (PE busy ~304 us at 90%,
   ACT ~273 us; bf16-everywhere baseline with (batch x q-half) sharding
   was 424.7 us).

dtypes: fp8 matmul inputs for projections, bf16 for scores/PV (fp8 noise
there fails the 2e-2 gate: softmax-weight noise does not average out),
PSUM accumulation fp32. Expected end-to-end error ~7e-3 L2-relative.
bv/bo are folded into the output on the host (softmax rows sum to 1, so
bv contributes exactly Wo @ bv); bq/bk are added on device via K=1 bf16
matmuls (skipped when the biases are all-zero, as in this problem).
"""

import numpy as np
import ml_dtypes
from contextlib import ExitStack

import concourse.bacc as bacc
import concourse.bass as bass
import concourse.tile as tile
import concourse.mybir as mybir
from concourse.bass_utils import run_bass_kernel_spmd

F8 = mybir.dt.float8e4
BF16 = mybir.dt.bfloat16
F32 = mybir.dt.float32
DR = mybir.MatmulPerfMode.DoubleRow

B, S, D, H = 4, 2048, 1024, 16
HD = 64
NCORE = 8
EH = 512           # e-columns (4 head-pairs) owned per core
NHP = 4            # head pairs per core
WSC = 32.0         # host-side weight scale (fp8 subnormal avoidance)
CSC = 32.0         # ctx scale for the out-projection hi/lo split
NP_F8 = ml_dtypes.float8_e4m3
NP_BF16 = ml_dtypes.bfloat16

_EO_MASK = [x for i in range(16) for x in (2 * i + 1, 2 * i)]


def _build_kernel(with_bias=True, dump=False):
    nc = bacc.Bacc("TRN2", target_bir_lowering=False, debug=False,
                   num_devices=NCORE)
    if dump:
        qt_d = nc.dram_tensor("qt", [128, NHP, S], BF16, kind="ExternalOutput")
        kt_d = nc.dram_tensor("ktd", [128, NHP, S], BF16, kind="ExternalOutput")
        v_d = nc.dram_tensor("vd", [128, 16, NHP, 131], BF16,
                             kind="ExternalOutput")
        ctx_d = nc.dram_tensor("ctxd", [128, NHP, S], BF16,
                               kind="ExternalOutput")

    xh_d = nc.dram_tensor("xh", [D, S], F8, kind="ExternalInput")
    xl_d = nc.dram_tensor("xl", [D, S], F8, kind="ExternalInput")
    wqh_d = nc.dram_tensor("wqh", [D, EH], F8, kind="ExternalInput")
    wql_d = nc.dram_tensor("wql", [D, EH], F8, kind="ExternalInput")
    wkh_d = nc.dram_tensor("wkh", [D, EH], F8, kind="ExternalInput")
    wkl_d = nc.dram_tensor("wkl", [D, EH], F8, kind="ExternalInput")
    wvh_d = nc.dram_tensor("wvh", [D, EH], F8, kind="ExternalInput")
    wvl_d = nc.dram_tensor("wvl", [D, EH], F8, kind="ExternalInput")
    woh_d = nc.dram_tensor("woh", [NHP, 128, D], F8, kind="ExternalInput")
    wol_d = nc.dram_tensor("wol", [NHP, 128, D], F8, kind="ExternalInput")
    bq_d = nc.dram_tensor("bq", [1, EH], BF16, kind="ExternalInput")
    bk_d = nc.dram_tensor("bk", [1, EH], BF16, kind="ExternalInput")
    cc_d = nc.dram_tensor("cc", [128, S], BF16, kind="ExternalInput")
    sg_d = nc.dram_tensor("sg", [128, S], BF16, kind="ExternalInput")
    out_d = nc.dram_tensor("out", [S, D], BF16, kind="ExternalOutput")

    with tile.TileContext(nc) as tc, ExitStack() as ex:
        const_p = ex.enter_context(tc.tile_pool(name="const", bufs=1))
        qk_p = ex.enter_context(tc.tile_pool(name="qk", bufs=1))
        sw_p = ex.enter_context(tc.tile_pool(name="sw", bufs=2))
        exp_p = ex.enter_context(tc.tile_pool(name="expp", bufs=12))
        rz_p = ex.enter_context(tc.tile_pool(name="rz", bufs=3))
        out_p = ex.enter_context(tc.tile_pool(name="outp", bufs=2))
        # PSUM (8 banks of [128, 2KB]):
        #   proj 2 x [128,512] = 2 banks, scores 2 x [128,1024] = 4 banks,
        #   ctx cA/cB [65,512] = 2 banks
        ps_proj = ex.enter_context(tc.tile_pool(name="psp", bufs=2, space="PSUM"))
        ps_sc = ex.enter_context(tc.tile_pool(name="pssc", bufs=2, space="PSUM"))
        ps_ctx = ex.enter_context(tc.tile_pool(name="psctx", bufs=1, space="PSUM"))

        # ---- resident tensors (DMA emission order = criticality order) ----
        xh_sb = const_p.tile([128, 8, S], F8)  # [d%128, d//128, s]
        xl_sb = const_p.tile([128, 8, S], F8)
        xh_re = xh_d.ap().rearrange("(dt p) s -> p dt s", p=128)
        xl_re = xl_d.ap().rearrange("(dt p) s -> p dt s", p=128)
        def load_w(dram, name):
            sb = const_p.tile([128, 8, EH], F8, name=name)
            re = dram.ap().rearrange("(dt p) e -> p dt e", p=128)
            nc.sync.dma_start(out=sb[:], in_=re)
            return sb

        cc_sb = const_p.tile([128, S], BF16)
        sg_sb = const_p.tile([128, S], BF16)
        wkh_sb = load_w(wkh_d, "wkh")
        nc.sync.dma_start(out=xh_sb[:, :, 0:512], in_=xh_re[:, :, 0:512])
        wkl_sb = load_w(wkl_d, "wkl")
        nc.sync.dma_start(out=xl_sb[:, :, 0:512], in_=xl_re[:, :, 0:512])
        nc.sync.dma_start(out=cc_sb[:, 0:512], in_=cc_d.ap()[:, 0:512])
        nc.sync.dma_start(out=sg_sb[:, 0:512], in_=sg_d.ap()[:, 0:512])
        # Q weights for pair 0 only; remaining pairs stream later
        wqh_sb = const_p.tile([128, 8, EH], F8, name="wqh")
        wql_sb = const_p.tile([128, 8, EH], F8, name="wql")
        wqh_re = wqh_d.ap().rearrange("(dt p) e -> p dt e", p=128)
        wql_re = wql_d.ap().rearrange("(dt p) e -> p dt e", p=128)
        nc.sync.dma_start(out=wqh_sb[:, :, 0:128], in_=wqh_re[:, :, 0:128])
        nc.sync.dma_start(out=wql_sb[:, :, 0:128], in_=wql_re[:, :, 0:128])
        nc.sync.dma_start(out=xh_sb[:, :, 512:1024], in_=xh_re[:, :, 512:1024])
        nc.sync.dma_start(out=xl_sb[:, :, 512:1024], in_=xl_re[:, :, 512:1024])
        wvh_sb = load_w(wvh_d, "wvh")
        wvl_sb = load_w(wvl_d, "wvl")
        nc.sync.dma_start(out=cc_sb[:, 512:1024], in_=cc_d.ap()[:, 512:1024])
        nc.sync.dma_start(out=sg_sb[:, 512:1024], in_=sg_d.ap()[:, 512:1024])
        nc.sync.dma_start(out=xh_sb[:, :, 1024:1536], in_=xh_re[:, :, 1024:1536])
        nc.sync.dma_start(out=xl_sb[:, :, 1024:1536], in_=xl_re[:, :, 1024:1536])
        nc.sync.dma_start(out=cc_sb[:, 1024:], in_=cc_d.ap()[:, 1024:])
        nc.sync.dma_start(out=sg_sb[:, 1024:], in_=sg_d.ap()[:, 1024:])
        nc.sync.dma_start(out=xh_sb[:, :, 1536:2048], in_=xh_re[:, :, 1536:2048])
        nc.sync.dma_start(out=xl_sb[:, :, 1536:2048], in_=xl_re[:, :, 1536:2048])
        nc.sync.dma_start(out=wqh_sb[:, :, 128:], in_=wqh_re[:, :, 128:])
        nc.sync.dma_start(out=wql_sb[:, :, 128:], in_=wql_re[:, :, 128:])

        woh_sb = const_p.tile([128, NHP, D], F8)
        nc.sync.dma_start(out=woh_sb[:],
                          in_=woh_d.ap().rearrange("g p e -> p g e"))
        wol_sb = const_p.tile([128, NHP, D], F8)
        nc.sync.dma_start(out=wol_sb[:],
                          in_=wol_d.ap().rearrange("g p e -> p g e"))

        if with_bias:
            bq_sb = const_p.tile([1, EH], BF16)
            nc.sync.dma_start(out=bq_sb[:], in_=bq_d.ap())
            bk_sb = const_p.tile([1, EH], BF16)
            nc.sync.dma_start(out=bk_sb[:], in_=bk_d.ap())
            ones_bf = const_p.tile([1, 512], BF16)
            nc.vector.memset(ones_bf[:], 1.0)
        else:
            bq_sb = bk_sb = None

        qt = qk_p.tile([128, NHP, S], BF16)   # Q^T, all head pairs
        kt = qk_p.tile([128, NHP, S], BF16)   # K^T
        # v_sb[kt][pq]: [0:64]=headA, col 64(,65)=ones, 66:130=headB, 130=one
        v_sb = qk_p.tile([128, 16, NHP, 131], BF16)
        nc.gpsimd.memset(v_sb[:, :, :, 64:66], 1.0)
        nc.gpsimd.memset(v_sb[:, :, :, 130:131], 1.0)
        ctx_sb = qk_p.tile([128, NHP, S], BF16)   # packed ctx^T
        # CSC-scaled hi/lo splits, one tile per group PAIR so the out-proj's
        # group-01 matmuls carry no (coarse-AP) false dep on group-23 splits
        ctx_hi = [qk_p.tile([128, 2, S], F8, name=f"cxh{i}") for i in range(2)]
        ctx_lo = [qk_p.tile([128, 2, S], F8, name=f"cxl{i}") for i in range(2)]

        def dr_mms(p_ps, terms, first_start=True, last_stop=True):
            """12 DoubleRow matmuls accumulating 3 compensation terms."""
            n = 4 * len(terms)
            i = 0
            for (l_sb, r_aps) in terms:
                for dtp in range(4):
                    nc.tensor.matmul(
                        p_ps, l_sb(dtp), r_aps(dtp),
                        start=(first_start and i == 0),
                        stop=(last_stop and i == n - 1), perf_mode=DR)
                    i += 1

        def proj_qk(wh_sb, wl_sb, b_sb, hp, ch, dst):
            """dst[:, hp, ch*512...] = rot-able (x @ W^T)^T rows of pair hp."""
            cs = bass.ts(ch, 512)
            p_t = ps_proj.tile([128, 512], F32, tag="proj", name="projp")
            p_ps = p_t[:]
            es = bass.ts(hp, 128)
            dr_mms(p_ps, [
                (lambda dtp, w=wh_sb: w[:, 2 * dtp:2 * dtp + 2, es],
                 lambda dtp: xh_sb[:, 2 * dtp:2 * dtp + 2, cs]),
                (lambda dtp, w=wl_sb: w[:, 2 * dtp:2 * dtp + 2, es],
                 lambda dtp: xh_sb[:, 2 * dtp:2 * dtp + 2, cs]),
                (lambda dtp, w=wh_sb: w[:, 2 * dtp:2 * dtp + 2, es],
                 lambda dtp: xl_sb[:, 2 * dtp:2 * dtp + 2, cs]),
            ], last_stop=not with_bias)
            if with_bias:
                nc.tensor.matmul(p_ps, b_sb[0:1, es], ones_bf[0:1, :],
                                 start=False, stop=True, skip_group_check=True)
            nc.vector.tensor_scalar_mul(dst[:, hp, cs], p_ps, 1.0 / WSC)

        def rope(dst, hp, ch):
            """in-place rotate dst[:, hp, ch*512...]; sw scratch."""
            cs = bass.ts(ch, 512)
            sw = sw_p.tile([128, 512], BF16, tag="sw")
            nc.vector.stream_shuffle(sw[:], dst[:, hp, cs], _EO_MASK)
            nc.vector.tensor_mul(sw[:], sw[:], sg_sb[:, cs])
            nc.vector.tensor_mul(dst[:, hp, cs], dst[:, hp, cs], cc_sb[:, cs])
            nc.vector.tensor_add(dst[:, hp, cs], dst[:, hp, cs], sw[:])

        def v_tile(st, half):
            """v_sb[:, st, 2*half:2*half+2] <- (x_st^T @ Wv)/WSC, 2 pairs.

            Split by pair-half so only pairs 0-1 are produced in the
            (DMA/PE-crowded) first block; pairs 2-3 spread as fillers."""
            v_t = ps_proj.tile([128, 512], F32, tag="proj", name="vp")
            v_ps = v_t[:, 0:256]
            ss = bass.ts(st, 128)
            hs = slice(half * 256, half * 256 + 256)
            dr_mms(v_ps, [
                (lambda dtp: xh_sb[:, 2 * dtp:2 * dtp + 2, ss],
                 lambda dtp: wvh_sb[:, 2 * dtp:2 * dtp + 2, hs]),
                (lambda dtp: xl_sb[:, 2 * dtp:2 * dtp + 2, ss],
                 lambda dtp: wvh_sb[:, 2 * dtp:2 * dtp + 2, hs]),
                (lambda dtp: xh_sb[:, 2 * dtp:2 * dtp + 2, ss],
                 lambda dtp: wvl_sb[:, 2 * dtp:2 * dtp + 2, hs]),
            ])
            vdst = v_sb[:, st, 2 * half, :]
            dst_ap = bass.AP(tensor=vdst.tensor, offset=vdst.offset,
                             ap=[vdst.ap[0], [131, 2], [66, 2], [1, 64]])
            nc.vector.tensor_scalar_mul(
                dst_ap,
                v_ps.rearrange("p (pq j e) -> p pq j e", pq=2, j=2),
                1.0 / WSC)

        carry = [None]  # deferred (last-PV, normalize) of the previous block

        def attention(hp, qc, emit_mid=None):
            """2 heads of pair hp for query chunk qc (512 q).

            The key loop is software-pipelined: PV runs one kt2 iteration
            behind scores/exp, so the in-order PE stream never waits on the
            Activation engine (the next scores pair sits between an exp and
            the PV that consumes it)."""
            qs = bass.ts(qc, 512)
            cA = ps_ctx.tile([65, 512], F32, tag="cA")
            cB = ps_ctx.tile([65, 512], F32, tag="cB")

            def pv(eA, eB, kt2):
                for j in range(2):
                    k2 = kt2 * 2 + j
                    nc.tensor.matmul(cA[:], v_sb[:, k2, hp, 0:65],
                                     eA[:, bass.ts(j, 512)],
                                     start=(k2 == 0), stop=(k2 == 15))
                    nc.tensor.matmul(cB[:], v_sb[:, k2, hp, 66:131],
                                     eB[:, bass.ts(j, 512)],
                                     start=(k2 == 0), stop=(k2 == 15))

            def finalize(on_act=False):
                # normalize into resident ctx^T + hi/lo split for out-proj.
                # The evacuation copy frees the PSUM bank after one fast op,
                # not the whole recip/bcast/mul chain; the final flush runs
                # it on the by-then-idle Activation engine.
                evs, rzs, rbss = [], [], []
                for cps in (cA, cB):
                    ev = rz_p.tile([65, 512], F32, tag="ev")
                    if on_act:
                        nc.scalar.activation(
                            ev[:], cps[:],
                            mybir.ActivationFunctionType.Copy)
                    else:
                        nc.vector.tensor_copy(ev[:], cps[:])
                    evs.append(ev)
                for ev in evs:
                    rz = rz_p.tile([1, 512], F32, tag="rz")
                    nc.vector.reciprocal(rz[0:1, :], ev[64:65, :])
                    rzs.append(rz)
                for rz in rzs:
                    rbs = rz_p.tile([HD, 512], F32, tag="rbs")
                    nc.gpsimd.partition_broadcast(rbs[:], rz[0:1, :])
                    rbss.append(rbs)
                for hh in range(2):
                    nc.vector.tensor_mul(ctx_sb[bass.ts(hh, HD), hp, qs],
                                         evs[hh][0:64, :], rbss[hh][:])
                hi = ctx_hi[hp // 2][:, hp % 2, qs]
                lo = ctx_lo[hp // 2][:, hp % 2, qs]
                nc.vector.tensor_scalar_mul(hi, ctx_sb[:, hp, qs], CSC)
                nc.vector.scalar_tensor_tensor(lo, ctx_sb[:, hp, qs], CSC,
                                               hi, mybir.AluOpType.mult,
                                               mybir.AluOpType.subtract)

            pend = None
            for kt2 in range(8):
                sA = ps_sc.tile([128, 1024], F32, tag="sA")
                sB = ps_sc.tile([128, 1024], F32, tag="sA", name="sB")
                for j in range(2):
                    k2 = kt2 * 2 + j
                    nc.tensor.matmul(sA[:, bass.ts(j, 512)],
                                     kt[0:64, hp, bass.ts(k2, 128)],
                                     qt[0:64, hp, qs], start=True, stop=True)
                    nc.tensor.matmul(sB[:, bass.ts(j, 512)],
                                     kt[64:128, hp, bass.ts(k2, 128)],
                                     qt[64:128, hp, qs], start=True, stop=True)
                eA = exp_p.tile([128, 1024], BF16, tag="e")
                nc.scalar.activation(eA[:], sA[:],
                                     mybir.ActivationFunctionType.Exp,
                                     scale=0.125)
                eB = exp_p.tile([128, 1024], BF16, tag="e")
                nc.scalar.activation(eB[:], sB[:],
                                     mybir.ActivationFunctionType.Exp,
                                     scale=0.125)
                if kt2 == 0 and carry[0] is not None:
                    # previous block's last PV + its normalize, emitted here
                    # so the exp pipeline covers the PV's wait. MUST precede
                    # emit_mid: hp3's out-proj fillers read the ctx splits
                    # this writes, and the tile framework cannot order a
                    # read emitted before its writer.
                    prev_pv, prev_fin = carry[0]
                    prev_pv()
                    prev_fin()
                    carry[0] = None
                if emit_mid is not None:
                    emit_mid(kt2)
                if pend is not None:
                    pv(*pend)
                pend = (eA, eB, kt2)
            lp = pend
            carry[0] = ((lambda: pv(*lp)), finalize)

        def out_proj(st, on_act=False):
            """out[st*128 ...] = (ctx^T).T @ Wo' / (WSC*CSC).

            on_act (final flush): copies ride the idle ACT engine and the
            psum comes from the by-then-idle scores ring."""
            o_sb = out_p.tile([128, D], BF16, tag="ot")
            ss = bass.ts(st, 128)
            if on_act:
                o_t2 = ps_sc.tile([128, 1024], F32, tag="sA", name="opf")
            for ec in range(2):
                if on_act:
                    o_ps = o_t2[:, bass.ts(ec, 512)]
                else:
                    o_t = ps_proj.tile([128, 512], F32, tag="proj", name="op")
                    o_ps = o_t[:]
                es = bass.ts(ec, 512)
                # two psum groups (one per group pair) so the gp0 matmuls
                # carry no sem-wait on the freshest ctx split (group input
                # waits attach to each group's first instruction)
                for gp in range(2):
                    i = 0
                    for (c_sb, w_sb) in ((ctx_hi[gp], woh_sb),
                                         (ctx_hi[gp], wol_sb),
                                         (ctx_lo[gp], woh_sb)):
                        nc.tensor.matmul(
                            o_ps, c_sb[:, :, ss],
                            w_sb[:, 2 * gp:2 * gp + 2, es],
                            start=(gp == 0 and i == 0), stop=(i == 2),
                            perf_mode=DR, skip_group_check=(gp == 1))
                        i += 1
                if on_act:
                    nc.scalar.activation(o_sb[:, es], o_ps,
                                         mybir.ActivationFunctionType.Copy,
                                         scale=1.0 / (WSC * CSC))
                else:
                    nc.vector.tensor_scalar_mul(o_sb[:, es], o_ps,
                                                1.0 / (WSC * CSC))
                nc.sync.dma_start(out=out_d.ap()[ss, es], in_=o_sb[:, es])

        # ---- emission ----
        # Attention(0, 0) starts right after K/Q chunk 0; everything else
        # (remaining K chunks, V tiles, later pairs' projections, the
        # out-projection) is spread as PE filler inside the attention key
        # loops so the in-order PE stream never stalls on the exp pipeline
        # or the PSUM-accumulator recycle.
        def q_chunk(hp, ch):
            proj_qk(wqh_sb, wql_sb, bq_sb, hp, ch, qt)
            rope(qt, hp, ch)

        def k_chunk(hp, ch):
            proj_qk(wkh_sb, wkl_sb, bk_sb, hp, ch, kt)
            rope(kt, hp, ch)

        k_chunk(0, 0)
        q_chunk(0, 0)

        def first_mid(kt2):
            # V tiles lag one iteration behind scores so the exp pipeline
            # fills before the V-production burst; the last two spill into
            # the next block (whose carried PV consumes them).
            if kt2 < 3:
                k_chunk(0, kt2 + 1)
            elif kt2 == 3:
                q_chunk(0, 1)
            if kt2 >= 1:
                v_tile(2 * (kt2 - 1), 0)
                v_tile(2 * kt2 - 1, 0)
            if kt2 == 7:
                v_tile(14, 0)
                v_tile(15, 0)

        # filler lists per (hp, qc), fired at evenly-spread kt2 positions
        def vh1(a, b):
            return lambda: (v_tile(a, 1), v_tile(b, 1))

        fillers = {
            (0, 1): [lambda: q_chunk(0, 2), lambda: k_chunk(1, 0),
                     lambda: q_chunk(1, 0), vh1(0, 1)],
            (0, 2): [lambda: q_chunk(0, 3), lambda: k_chunk(1, 1),
                     lambda: q_chunk(1, 1), vh1(2, 3)],
            (0, 3): [lambda: k_chunk(1, 2), lambda: k_chunk(1, 3),
                     lambda: q_chunk(1, 2), vh1(4, 5)],
            (1, 0): [lambda: q_chunk(1, 3), lambda: k_chunk(2, 0),
                     lambda: q_chunk(2, 0), vh1(6, 7)],
            (1, 1): [lambda: k_chunk(2, 1), lambda: q_chunk(2, 1),
                     vh1(8, 9)],
            (1, 2): [lambda: k_chunk(2, 2), lambda: q_chunk(2, 2),
                     vh1(10, 11)],
            (1, 3): [lambda: k_chunk(2, 3), lambda: q_chunk(2, 3),
                     vh1(12, 13)],
            (2, 0): [lambda: k_chunk(3, 0), lambda: q_chunk(3, 0),
                     vh1(14, 15)],
            (2, 1): [lambda: k_chunk(3, 1), lambda: k_chunk(3, 2)],
            (2, 2): [lambda: k_chunk(3, 3), lambda: q_chunk(3, 1)],
            (2, 3): [lambda: q_chunk(3, 2)],
            (3, 0): [lambda: q_chunk(3, 3)],
            (3, 1): [lambda s=s: out_proj(s) for s in range(0, 4)],
            (3, 2): [lambda s=s: out_proj(s) for s in range(4, 8)],
            (3, 3): [lambda s=s: out_proj(s) for s in range(8, 12)],
        }

        def make_mid(items):
            n = len(items)
            pos = [(k * 8) // n for k in range(n)] if n else []
            idx = [0]

            def mid(kt2):
                while idx[0] < n and pos[idx[0]] <= kt2:
                    items[idx[0]]()
                    idx[0] += 1
            return mid

        for hp in range(NHP):
            for qc in range(4):
                if (hp, qc) == (0, 0):
                    attention(hp, qc, emit_mid=first_mid)
                else:
                    attention(hp, qc,
                              emit_mid=make_mid(fillers.get((hp, qc), [])))
        last_pv, last_fin = carry[0]
        last_pv()
        last_fin(on_act=True)
        carry[0] = None
        for st in range(12, 16):
            out_proj(st, on_act=(st % 2 == 1))
        if dump:
            nc.sync.dma_start(out=qt_d.ap(), in_=qt[:])
            nc.sync.dma_start(out=kt_d.ap(), in_=kt[:])
            nc.sync.dma_start(out=v_d.ap(), in_=v_sb[:])
            nc.sync.dma_start(out=ctx_d.ap(), in_=ctx_sb[:])

    nc.finalize()
    return nc


_NC = {}


def _get_nc(with_bias=True):
    if with_bias not in _NC:
        _NC[with_bias] = _build_kernel(with_bias)
    return _NC[with_bias]


def _f8_split(a):
    hi = a.astype(NP_F8)
    lo = (a - hi.astype(np.float32)).astype(NP_F8)
    return hi, lo


def _host_prep(hidden_states, Wq, bq, Wk, bk, Wv, bv, Wo, bo):
    """Build per-core input maps (host does layout transforms only)."""
    f32 = np.float32
    hidden_states = np.asarray(hidden_states, f32)
    Wq, Wk, Wv, Wo = (np.asarray(w, f32) for w in (Wq, Wk, Wv, Wo))
    bq, bk, bv, bo = (np.asarray(b, f32) for b in (bq, bk, bv, bo))

    # rope interleave: new row 64*blk + 2*i + t <- old row 64*blk + 32*t + i
    p = np.arange(D)
    blk, r = p // HD, p % HD
    perm = blk * HD + (r % 2) * 32 + (r // 2)

    # per-half weight slices, scaled by WSC, transposed, fp8 hi/lo split
    halves = []
    for hh in range(2):
        rows = slice(hh * EH, (hh + 1) * EH)
        wq_t = np.ascontiguousarray((Wq[perm][rows] * WSC).T)  # [D, EH]
        wk_t = np.ascontiguousarray((Wk[perm][rows] * WSC).T)
        wv_t = np.ascontiguousarray((Wv[rows] * WSC).T)
        wo_t = np.ascontiguousarray((Wo.T[rows] * WSC).reshape(NHP, 128, D))
        m = {}
        for nm, arr in (("wq", wq_t), ("wk", wk_t), ("wv", wv_t),
                        ("wo", wo_t)):
            m[nm + "h"], m[nm + "l"] = _f8_split(arr)
        m["bq"] = (bq[perm][rows] * WSC).reshape(1, EH).astype(NP_BF16)
        m["bk"] = (bk[perm][rows] * WSC).reshape(1, EH).astype(NP_BF16)
        halves.append(m)

    # rope tables (reference quirk: "c" is sin, "s" is cos), interleaved rows
    inv_freq = 1.0 / (10000.0 ** (np.arange(0, HD, 2, dtype=f32) / HD))
    ang = np.arange(S, dtype=f32)[:, None] * inv_freq[None, :]  # [S, 32]
    sin_t, cos_t = np.sin(ang), np.cos(ang)
    rows = np.arange(128)
    i_of = (rows % HD) // 2
    sign = np.where(rows % 2 == 0, -1.0, 1.0)
    cc = np.ascontiguousarray(sin_t.T[i_of, :]).astype(NP_BF16)        # [128, S]
    sg = np.ascontiguousarray(cos_t.T[i_of, :] * sign[:, None]).astype(NP_BF16)

    in_maps = []
    for c in range(NCORE):
        b_i, hh = c // 2, c % 2
        xt = np.ascontiguousarray(hidden_states[b_i].T)  # [D, S]
        xh, xl = _f8_split(xt)
        m = {"xh": xh, "xl": xl, "cc": cc, "sg": sg}
        m.update(halves[hh])
        in_maps.append(m)
    out_const = (Wo @ bv + bo).astype(f32)
    return in_maps, out_const


def kernel(hidden_states, Wq, bq, Wk, bk, Wv, bv, Wo, bo, _trace=False):
    in_maps, out_const = _host_prep(hidden_states, Wq, bq, Wk, bk, Wv, bv,
                                    Wo, bo)
    with_bias = bool(np.any(np.asarray(bq)) or np.any(np.asarray(bk)))
    nc = _get_nc(with_bias)
    res = run_bass_kernel_spmd(nc, in_maps, core_ids=list(range(NCORE)),
                               trace=_trace)
    out = np.empty((B, S, D), np.float32)
    for b_i in range(B):
        out[b_i] = np.asarray(res.results[2 * b_i]["out"], np.float32)
        out[b_i] += np.asarray(res.results[2 * b_i + 1]["out"], np.float32)
    out += out_const[None, None, :]
    if _trace:
        return out, res
    return out


# revision 51
# speedup vs baseline: 1.0271x; 1.0271x over previous
"""Trainium2 Bass kernel for nn_MultiHeadAttention_89318139888179.

Problem: B=4, S=2048, D=1024, H=16 heads (hd=64) fp32 multi-head attention
with (quirky) RoPE, y = softmax((rot(q) @ rot(k)^T)/8) v, projections are
x @ W^T + b with W [e,d].

Sharding: 8 cores = 4 batches x 2 head-halves. Each core computes Q/K/V for
its 8 heads over the batch's full 2048 queries, attention per-head local,
and a row-sharded out-projection partial (contraction over its 512 ctx
dims). The host sums each batch pair's partials during unshard (allowed:
kernel() takes full inputs / returns the full output; no collectives).
This halves the K/V projection work vs (batch x query-half) sharding,
which computed every batch's K/V twice.

Per-core layout (all matmul contractions on SBUF partitions):
 - Projections run in fp8(e4m3) DoubleRow perf mode (2 contraction tiles
   per pass at 0.5 cycles/row) with residual compensation:
   y = Wh.xh + Wh.xl + Wl.xh where t = t_hi + t_lo is an fp8 hi/lo split
   (hi = fp8(t), lo = fp8(t - hi)); the dropped Wl.xl term is O(eps^2).
   W rows are host-scaled by 32 so sigma~1 (raw sigma=1/32 sits in e4m3's
   subnormal flush zone); the PSUM->SBUF copy rescales by 1/32. Measured
   per-projection error ~0.13%, slightly better than bf16. 25% fewer
   PE column-passes than bf16 (6 vs 8 passes per 512 contraction).
 - Host interleaves Wq/Wk output rows so RoPE rotation pairs sit on
   adjacent (even,odd) partitions: rotation = DVE stream_shuffle + 2 mul
   + add against sin/cos tables.
 - scoresT[k, q] = K^T.T @ Q^T per head (bf16, K=64), exp on the scalar
   engine (scale=1/8 fused, no max subtraction: |logits| < ~6), PV as
   ctx^T = V_aug.T @ expT with a ones column so PSUM row 64 of the ctx
   accumulator is the softmax denominator (bf16: the M=65 row is free,
   matmul cost depends only on the moving dim).
 - 1/Z on the DVE (reciprocal), broadcast across partitions by the idle
   GPSIMD, normalize on DVE -- keeps the Activation engine exp-only
   (it is the second-busiest engine at ~0.9x the PE).
 - Normalized ctx^T head-pairs pack into a resident [128, 4, 2048] tile,
   are hi/lo split (x32 scale, one tile per group pair -- avoids coarse-AP
   false deps) for the fp8 out-projection; output lands [s, e] bf16 (psum
   copy rescales by 2^-10) and DMAs straight out.
 - Schedule: the attention key loop is software-pipelined (PV one kt2
   behind scores/exp; each block's last PV + normalize carried into the
   next block) and all projection / V / out-proj work is spread as PE
   filler inside the key loops, so the in-order PE stream never waits on
   the exp pipeline; fillers fire in the back half of each key loop
   (off=4) so the exp stream gets engine priority up front. Weight DMAs
   are sliced by first use (pair-0 K/Q columns and pair-01 V columns
   lead) to cut the startup critical bytes. Output is bf16 (host sums
   the pair partials in fp32; 4-deep output staging ring so the flush
   copies never wait on earlier tiles' DMA semaphores). TimelineSim:
   ~329.4 us (PE busy ~301 us; ACT ~273 us; bf16-everywhere baseline
   with (batch x q-half) sharding was 424.7 us).

dtypes: fp8 matmul inputs for projections, bf16 for scores/PV (fp8 noise
there fails the 2e-2 gate: softmax-weight noise does not average out),
PSUM accumulation fp32. Expected end-to-end error ~7e-3 L2-relative.
bv/bo are folded into the output on the host (softmax rows sum to 1, so
bv contributes exactly Wo @ bv); bq/bk are added on device via K=1 bf16
matmuls (skipped when the biases are all-zero, as in this problem).
"""

import numpy as np
import ml_dtypes
from contextlib import ExitStack

import concourse.bacc as bacc
import concourse.bass as bass
import concourse.tile as tile
import concourse.mybir as mybir
from concourse.bass_utils import run_bass_kernel_spmd

F8 = mybir.dt.float8e4
BF16 = mybir.dt.bfloat16
F32 = mybir.dt.float32
DR = mybir.MatmulPerfMode.DoubleRow

B, S, D, H = 4, 2048, 1024, 16
HD = 64
NCORE = 8
EH = 512           # e-columns (4 head-pairs) owned per core
NHP = 4            # head pairs per core
WSC = 32.0         # host-side weight scale (fp8 subnormal avoidance)
CSC = 32.0         # ctx scale for the out-projection hi/lo split
NP_F8 = ml_dtypes.float8_e4m3
NP_BF16 = ml_dtypes.bfloat16

_EO_MASK = [x for i in range(16) for x in (2 * i + 1, 2 * i)]


def _build_kernel(with_bias=True, dump=False):
    nc = bacc.Bacc("TRN2", target_bir_lowering=False, debug=False,
                   num_devices=NCORE)
    if dump:
        qt_d = nc.dram_tensor("qt", [128, NHP, S], BF16, kind="ExternalOutput")
        kt_d = nc.dram_tensor("ktd", [128, NHP, S], BF16, kind="ExternalOutput")
        v_d = nc.dram_tensor("vd", [128, 16, NHP, 131], BF16,
                             kind="ExternalOutput")
        ctx_d = nc.dram_tensor("ctxd", [128, NHP, S], BF16,
                               kind="ExternalOutput")

    xh_d = nc.dram_tensor("xh", [D, S], F8, kind="ExternalInput")
    xl_d = nc.dram_tensor("xl", [D, S], F8, kind="ExternalInput")
    wqh_d = nc.dram_tensor("wqh", [D, EH], F8, kind="ExternalInput")
    wql_d = nc.dram_tensor("wql", [D, EH], F8, kind="ExternalInput")
    wkh_d = nc.dram_tensor("wkh", [D, EH], F8, kind="ExternalInput")
    wkl_d = nc.dram_tensor("wkl", [D, EH], F8, kind="ExternalInput")
    wvh_d = nc.dram_tensor("wvh", [D, EH], F8, kind="ExternalInput")
    wvl_d = nc.dram_tensor("wvl", [D, EH], F8, kind="ExternalInput")
    woh_d = nc.dram_tensor("woh", [NHP, 128, D], F8, kind="ExternalInput")
    wol_d = nc.dram_tensor("wol", [NHP, 128, D], F8, kind="ExternalInput")
    bq_d = nc.dram_tensor("bq", [1, EH], BF16, kind="ExternalInput")
    bk_d = nc.dram_tensor("bk", [1, EH], BF16, kind="ExternalInput")
    cc_d = nc.dram_tensor("cc", [128, S], BF16, kind="ExternalInput")
    sg_d = nc.dram_tensor("sg", [128, S], BF16, kind="ExternalInput")
    out_d = nc.dram_tensor("out", [S, D], BF16, kind="ExternalOutput")

    with tile.TileContext(nc) as tc, ExitStack() as ex:
        const_p = ex.enter_context(tc.tile_pool(name="const", bufs=1))
        qk_p = ex.enter_context(tc.tile_pool(name="qk", bufs=1))
        sw_p = ex.enter_context(tc.tile_pool(name="sw", bufs=2))
        exp_p = ex.enter_context(tc.tile_pool(name="expp", bufs=8))
        rz_p = ex.enter_context(tc.tile_pool(name="rz", bufs=3))
        out_p = ex.enter_context(tc.tile_pool(name="outp", bufs=4))
        # PSUM (8 banks of [128, 2KB]):
        #   proj 2 x [128,512] = 2 banks, scores 2 x [128,1024] = 4 banks,
        #   ctx cA/cB [65,512] = 2 banks
        ps_proj = ex.enter_context(tc.tile_pool(name="psp", bufs=2, space="PSUM"))
        ps_sc = ex.enter_context(tc.tile_pool(name="pssc", bufs=2, space="PSUM"))
        ps_ctx = ex.enter_context(tc.tile_pool(name="psctx", bufs=1, space="PSUM"))

        # ---- resident tensors (DMA emission order = criticality order) ----
        xh_sb = const_p.tile([128, 8, S], F8)  # [d%128, d//128, s]
        xl_sb = const_p.tile([128, 8, S], F8)
        xh_re = xh_d.ap().rearrange("(dt p) s -> p dt s", p=128)
        xl_re = xl_d.ap().rearrange("(dt p) s -> p dt s", p=128)
        def load_w(dram, name):
            sb = const_p.tile([128, 8, EH], F8, name=name)
            re = dram.ap().rearrange("(dt p) e -> p dt e", p=128)
            nc.sync.dma_start(out=sb[:], in_=re)
            return sb

        cc_sb = const_p.tile([128, S], BF16)
        sg_sb = const_p.tile([128, S], BF16)
        # first block needs only pair-0 K/Q weight columns and pair-01 V
        # columns: load those slices first, stream the rest behind x
        def w_tile(name):
            return const_p.tile([128, 8, EH], F8, name=name)

        wkh_sb, wkl_sb = w_tile("wkh"), w_tile("wkl")
        wqh_sb, wql_sb = w_tile("wqh"), w_tile("wql")
        wvh_sb, wvl_sb = w_tile("wvh"), w_tile("wvl")
        res_ = {}
        for nm, dram, sb in (("wkh", wkh_d, wkh_sb), ("wkl", wkl_d, wkl_sb),
                             ("wqh", wqh_d, wqh_sb), ("wql", wql_d, wql_sb),
                             ("wvh", wvh_d, wvh_sb), ("wvl", wvl_d, wvl_sb)):
            res_[nm] = dram.ap().rearrange("(dt p) e -> p dt e", p=128)
        nc.sync.dma_start(out=wkh_sb[:, :, 0:128], in_=res_["wkh"][:, :, 0:128])
        nc.sync.dma_start(out=xh_sb[:, :, 0:512], in_=xh_re[:, :, 0:512])
        nc.sync.dma_start(out=wkl_sb[:, :, 0:128], in_=res_["wkl"][:, :, 0:128])
        nc.sync.dma_start(out=xl_sb[:, :, 0:512], in_=xl_re[:, :, 0:512])
        nc.sync.dma_start(out=cc_sb[:, 0:512], in_=cc_d.ap()[:, 0:512])
        nc.sync.dma_start(out=sg_sb[:, 0:512], in_=sg_d.ap()[:, 0:512])
        nc.sync.dma_start(out=wqh_sb[:, :, 0:128], in_=res_["wqh"][:, :, 0:128])
        nc.sync.dma_start(out=wql_sb[:, :, 0:128], in_=res_["wql"][:, :, 0:128])
        nc.sync.dma_start(out=wvh_sb[:, :, 0:256], in_=res_["wvh"][:, :, 0:256])
        nc.sync.dma_start(out=wvl_sb[:, :, 0:256], in_=res_["wvl"][:, :, 0:256])
        nc.sync.dma_start(out=xh_sb[:, :, 512:1024], in_=xh_re[:, :, 512:1024])
        nc.sync.dma_start(out=xl_sb[:, :, 512:1024], in_=xl_re[:, :, 512:1024])
        nc.sync.dma_start(out=cc_sb[:, 512:1024], in_=cc_d.ap()[:, 512:1024])
        nc.sync.dma_start(out=sg_sb[:, 512:1024], in_=sg_d.ap()[:, 512:1024])
        nc.sync.dma_start(out=xh_sb[:, :, 1024:1536], in_=xh_re[:, :, 1024:1536])
        nc.sync.dma_start(out=xl_sb[:, :, 1024:1536], in_=xl_re[:, :, 1024:1536])
        nc.sync.dma_start(out=cc_sb[:, 1024:], in_=cc_d.ap()[:, 1024:])
        nc.sync.dma_start(out=sg_sb[:, 1024:], in_=sg_d.ap()[:, 1024:])
        nc.sync.dma_start(out=xh_sb[:, :, 1536:2048], in_=xh_re[:, :, 1536:2048])
        nc.sync.dma_start(out=xl_sb[:, :, 1536:2048], in_=xl_re[:, :, 1536:2048])
        nc.sync.dma_start(out=wkh_sb[:, :, 128:], in_=res_["wkh"][:, :, 128:])
        nc.sync.dma_start(out=wkl_sb[:, :, 128:], in_=res_["wkl"][:, :, 128:])
        nc.sync.dma_start(out=wqh_sb[:, :, 128:], in_=res_["wqh"][:, :, 128:])
        nc.sync.dma_start(out=wql_sb[:, :, 128:], in_=res_["wql"][:, :, 128:])
        nc.sync.dma_start(out=wvh_sb[:, :, 256:], in_=res_["wvh"][:, :, 256:])
        nc.sync.dma_start(out=wvl_sb[:, :, 256:], in_=res_["wvl"][:, :, 256:])

        woh_sb = const_p.tile([128, NHP, D], F8)
        nc.sync.dma_start(out=woh_sb[:],
                          in_=woh_d.ap().rearrange("g p e -> p g e"))
        wol_sb = const_p.tile([128, NHP, D], F8)
        nc.sync.dma_start(out=wol_sb[:],
                          in_=wol_d.ap().rearrange("g p e -> p g e"))

        if with_bias:
            bq_sb = const_p.tile([1, EH], BF16)
            nc.sync.dma_start(out=bq_sb[:], in_=bq_d.ap())
            bk_sb = const_p.tile([1, EH], BF16)
            nc.sync.dma_start(out=bk_sb[:], in_=bk_d.ap())
            ones_bf = const_p.tile([1, 512], BF16)
            nc.vector.memset(ones_bf[:], 1.0)
        else:
            bq_sb = bk_sb = None

        qt = qk_p.tile([128, NHP, S], BF16)   # Q^T, all head pairs
        kt = qk_p.tile([128, NHP, S], BF16)   # K^T
        # v_sb[kt][pq]: [0:64]=headA, col 64(,65)=ones, 66:130=headB, 130=one
        v_sb = qk_p.tile([128, 16, NHP, 131], BF16)
        nc.gpsimd.memset(v_sb[:, :, :, 64:66], 1.0)
        nc.gpsimd.memset(v_sb[:, :, :, 130:131], 1.0)
        ctx_sb = qk_p.tile([128, NHP, S], BF16)   # packed ctx^T
        # CSC-scaled hi/lo splits, one tile per group PAIR so the out-proj's
        # group-01 matmuls carry no (coarse-AP) false dep on group-23 splits
        ctx_hi = [qk_p.tile([128, 2, S], F8, name=f"cxh{i}") for i in range(2)]
        ctx_lo = [qk_p.tile([128, 2, S], F8, name=f"cxl{i}") for i in range(2)]

        def dr_mms(p_ps, terms, first_start=True, last_stop=True):
            """12 DoubleRow matmuls accumulating 3 compensation terms."""
            n = 4 * len(terms)
            i = 0
            for (l_sb, r_aps) in terms:
                for dtp in range(4):
                    nc.tensor.matmul(
                        p_ps, l_sb(dtp), r_aps(dtp),
                        start=(first_start and i == 0),
                        stop=(last_stop and i == n - 1), perf_mode=DR)
                    i += 1

        def proj_qk(wh_sb, wl_sb, b_sb, hp, ch, dst):
            """dst[:, hp, ch*512...] = rot-able (x @ W^T)^T rows of pair hp."""
            cs = bass.ts(ch, 512)
            p_t = ps_proj.tile([128, 512], F32, tag="proj", name="projp")
            p_ps = p_t[:]
            es = bass.ts(hp, 128)
            dr_mms(p_ps, [
                (lambda dtp, w=wh_sb: w[:, 2 * dtp:2 * dtp + 2, es],
                 lambda dtp: xh_sb[:, 2 * dtp:2 * dtp + 2, cs]),
                (lambda dtp, w=wl_sb: w[:, 2 * dtp:2 * dtp + 2, es],
                 lambda dtp: xh_sb[:, 2 * dtp:2 * dtp + 2, cs]),
                (lambda dtp, w=wh_sb: w[:, 2 * dtp:2 * dtp + 2, es],
                 lambda dtp: xl_sb[:, 2 * dtp:2 * dtp + 2, cs]),
            ], last_stop=not with_bias)
            if with_bias:
                nc.tensor.matmul(p_ps, b_sb[0:1, es], ones_bf[0:1, :],
                                 start=False, stop=True, skip_group_check=True)
            nc.vector.tensor_scalar_mul(dst[:, hp, cs], p_ps, 1.0 / WSC)

        def rope(dst, hp, ch):
            """in-place rotate dst[:, hp, ch*512...]; sw scratch."""
            cs = bass.ts(ch, 512)
            sw = sw_p.tile([128, 512], BF16, tag="sw")
            nc.vector.stream_shuffle(sw[:], dst[:, hp, cs], _EO_MASK)
            nc.vector.tensor_mul(sw[:], sw[:], sg_sb[:, cs])
            nc.vector.tensor_mul(dst[:, hp, cs], dst[:, hp, cs], cc_sb[:, cs])
            nc.vector.tensor_add(dst[:, hp, cs], dst[:, hp, cs], sw[:])

        def v_tile(st, half):
            """v_sb[:, st, 2*half:2*half+2] <- (x_st^T @ Wv)/WSC, 2 pairs.

            Split by pair-half so only pairs 0-1 are produced in the
            (DMA/PE-crowded) first block; pairs 2-3 spread as fillers."""
            v_t = ps_proj.tile([128, 512], F32, tag="proj", name="vp")
            v_ps = v_t[:, 0:256]
            ss = bass.ts(st, 128)
            hs = slice(half * 256, half * 256 + 256)
            dr_mms(v_ps, [
                (lambda dtp: xh_sb[:, 2 * dtp:2 * dtp + 2, ss],
                 lambda dtp: wvh_sb[:, 2 * dtp:2 * dtp + 2, hs]),
                (lambda dtp: xl_sb[:, 2 * dtp:2 * dtp + 2, ss],
                 lambda dtp: wvh_sb[:, 2 * dtp:2 * dtp + 2, hs]),
                (lambda dtp: xh_sb[:, 2 * dtp:2 * dtp + 2, ss],
                 lambda dtp: wvl_sb[:, 2 * dtp:2 * dtp + 2, hs]),
            ])
            vdst = v_sb[:, st, 2 * half, :]
            dst_ap = bass.AP(tensor=vdst.tensor, offset=vdst.offset,
                             ap=[vdst.ap[0], [131, 2], [66, 2], [1, 64]])
            nc.vector.tensor_scalar_mul(
                dst_ap,
                v_ps.rearrange("p (pq j e) -> p pq j e", pq=2, j=2),
                1.0 / WSC)

        carry = [None]  # deferred (last-PV, normalize) of the previous block

        def attention(hp, qc, emit_mid=None):
            """2 heads of pair hp for query chunk qc (512 q).

            The key loop is software-pipelined: PV runs one kt2 iteration
            behind scores/exp, so the in-order PE stream never waits on the
            Activation engine (the next scores pair sits between an exp and
            the PV that consumes it)."""
            qs = bass.ts(qc, 512)
            cA = ps_ctx.tile([65, 512], F32, tag="cA")
            cB = ps_ctx.tile([65, 512], F32, tag="cB")

            def pv(eA, eB, kt2):
                for j in range(2):
                    k2 = kt2 * 2 + j
                    nc.tensor.matmul(cA[:], v_sb[:, k2, hp, 0:65],
                                     eA[:, bass.ts(j, 512)],
                                     start=(k2 == 0), stop=(k2 == 15))
                    nc.tensor.matmul(cB[:], v_sb[:, k2, hp, 66:131],
                                     eB[:, bass.ts(j, 512)],
                                     start=(k2 == 0), stop=(k2 == 15))

            def finalize(on_act=False):
                # normalize into resident ctx^T + hi/lo split for out-proj.
                # The evacuation copy frees the PSUM bank after one fast op,
                # not the whole recip/bcast/mul chain; the final flush runs
                # it on the by-then-idle Activation engine.
                evs, rzs, rbss = [], [], []
                for cps in (cA, cB):
                    ev = rz_p.tile([65, 512], F32, tag="ev")
                    if on_act:
                        nc.scalar.activation(
                            ev[:], cps[:],
                            mybir.ActivationFunctionType.Copy)
                    else:
                        nc.vector.tensor_copy(ev[:], cps[:])
                    evs.append(ev)
                for ev in evs:
                    rz = rz_p.tile([1, 512], F32, tag="rz")
                    nc.vector.reciprocal(rz[0:1, :], ev[64:65, :])
                    rzs.append(rz)
                for rz in rzs:
                    rbs = rz_p.tile([HD, 512], F32, tag="rbs")
                    nc.gpsimd.partition_broadcast(rbs[:], rz[0:1, :])
                    rbss.append(rbs)
                for hh in range(2):
                    nc.vector.tensor_mul(ctx_sb[bass.ts(hh, HD), hp, qs],
                                         evs[hh][0:64, :], rbss[hh][:])
                hi = ctx_hi[hp // 2][:, hp % 2, qs]
                lo = ctx_lo[hp // 2][:, hp % 2, qs]
                nc.vector.tensor_scalar_mul(hi, ctx_sb[:, hp, qs], CSC)
                nc.vector.scalar_tensor_tensor(lo, ctx_sb[:, hp, qs], CSC,
                                               hi, mybir.AluOpType.mult,
                                               mybir.AluOpType.subtract)

            pend = None
            for kt2 in range(8):
                sA = ps_sc.tile([128, 1024], F32, tag="sA")
                sB = ps_sc.tile([128, 1024], F32, tag="sA", name="sB")
                for j in range(2):
                    k2 = kt2 * 2 + j
                    nc.tensor.matmul(sA[:, bass.ts(j, 512)],
                                     kt[0:64, hp, bass.ts(k2, 128)],
                                     qt[0:64, hp, qs], start=True, stop=True)
                    nc.tensor.matmul(sB[:, bass.ts(j, 512)],
                                     kt[64:128, hp, bass.ts(k2, 128)],
                                     qt[64:128, hp, qs], start=True, stop=True)
                eA = exp_p.tile([128, 1024], BF16, tag="e")
                nc.scalar.activation(eA[:], sA[:],
                                     mybir.ActivationFunctionType.Exp,
                                     scale=0.125)
                eB = exp_p.tile([128, 1024], BF16, tag="e")
                nc.scalar.activation(eB[:], sB[:],
                                     mybir.ActivationFunctionType.Exp,
                                     scale=0.125)
                if kt2 == 0 and carry[0] is not None:
                    # previous block's last PV + its normalize, emitted here
                    # so the exp pipeline covers the PV's wait. MUST precede
                    # emit_mid: hp3's out-proj fillers read the ctx splits
                    # this writes, and the tile framework cannot order a
                    # read emitted before its writer.
                    prev_pv, prev_fin = carry[0]
                    prev_pv()
                    prev_fin()
                    carry[0] = None
                if emit_mid is not None:
                    emit_mid(kt2)
                if pend is not None:
                    pv(*pend)
                pend = (eA, eB, kt2)
            lp = pend
            carry[0] = ((lambda: pv(*lp)), finalize)

        def out_proj(st, flush=False):
            """out[st*128 ...] = (ctx^T).T @ Wo' / (WSC*CSC).

            flush (the last 4 tiles): psum from the by-then-idle scores
            ring, copies alternate ACT/DVE per tile, one whole-tile DMA
            (fewer serialized HWDGE slots on the drain path)."""
            o_sb = out_p.tile([128, D], BF16, tag="ot")
            ss = bass.ts(st, 128)
            on_act = flush and (st % 2 == 1)
            if on_act:
                o_t2 = ps_sc.tile([128, 1024], F32, tag="sA", name="opf")
            for ec in range(2):
                if on_act:
                    o_ps = o_t2[:, bass.ts(ec, 512)]
                else:
                    o_t = ps_proj.tile([128, 512], F32, tag="proj", name="op")
                    o_ps = o_t[:]
                es = bass.ts(ec, 512)
                # two psum groups (one per group pair) so the gp0 matmuls
                # carry no sem-wait on the freshest ctx split (group input
                # waits attach to each group's first instruction)
                for gp in range(2):
                    i = 0
                    for (c_sb, w_sb) in ((ctx_hi[gp], woh_sb),
                                         (ctx_hi[gp], wol_sb),
                                         (ctx_lo[gp], woh_sb)):
                        nc.tensor.matmul(
                            o_ps, c_sb[:, :, ss],
                            w_sb[:, 2 * gp:2 * gp + 2, es],
                            start=(gp == 0 and i == 0), stop=(i == 2),
                            perf_mode=DR, skip_group_check=(gp == 1))
                        i += 1
                if on_act:
                    nc.scalar.activation(o_sb[:, es], o_ps,
                                         mybir.ActivationFunctionType.Copy,
                                         scale=1.0 / (WSC * CSC))
                else:
                    nc.vector.tensor_scalar_mul(o_sb[:, es], o_ps,
                                                1.0 / (WSC * CSC))
                nc.sync.dma_start(out=out_d.ap()[ss, es], in_=o_sb[:, es])

        # ---- emission ----
        # Attention(0, 0) starts right after K/Q chunk 0; everything else
        # (remaining K chunks, V tiles, later pairs' projections, the
        # out-projection) is spread as PE filler inside the attention key
        # loops so the in-order PE stream never stalls on the exp pipeline
        # or the PSUM-accumulator recycle.
        def q_chunk(hp, ch):
            proj_qk(wqh_sb, wql_sb, bq_sb, hp, ch, qt)
            rope(qt, hp, ch)

        def k_chunk(hp, ch):
            proj_qk(wkh_sb, wkl_sb, bk_sb, hp, ch, kt)
            rope(kt, hp, ch)

        k_chunk(0, 0)
        q_chunk(0, 0)

        def first_mid(kt2):
            # V tiles lag one iteration behind scores so the exp pipeline
            # fills before the V-production burst; the last two spill into
            # the next block (whose carried PV consumes them).
            if kt2 < 3:
                k_chunk(0, kt2 + 1)
            elif kt2 == 3:
                q_chunk(0, 1)
            if kt2 >= 1:
                v_tile(2 * (kt2 - 1), 0)
                v_tile(2 * kt2 - 1, 0)
            if kt2 == 7:
                v_tile(14, 0)
                v_tile(15, 0)

        # filler lists per (hp, qc), fired at evenly-spread kt2 positions
        def vh1(a, b):
            return lambda: (v_tile(a, 1), v_tile(b, 1))

        fillers = {
            (0, 1): [lambda: q_chunk(0, 2), lambda: k_chunk(1, 0),
                     lambda: q_chunk(1, 0), vh1(0, 1)],
            (0, 2): [lambda: q_chunk(0, 3), lambda: k_chunk(1, 1),
                     lambda: q_chunk(1, 1), vh1(2, 3)],
            (0, 3): [lambda: k_chunk(1, 2), lambda: k_chunk(1, 3),
                     lambda: q_chunk(1, 2), vh1(4, 5)],
            (1, 0): [lambda: q_chunk(1, 3), lambda: k_chunk(2, 0),
                     lambda: q_chunk(2, 0), vh1(6, 7)],
            (1, 1): [lambda: k_chunk(2, 1), lambda: q_chunk(2, 1),
                     vh1(8, 9)],
            (1, 2): [lambda: k_chunk(2, 2), lambda: q_chunk(2, 2),
                     vh1(10, 11)],
            (1, 3): [lambda: k_chunk(2, 3), lambda: q_chunk(2, 3),
                     vh1(12, 13)],
            (2, 0): [lambda: k_chunk(3, 0), lambda: q_chunk(3, 0),
                     vh1(14, 15)],
            (2, 1): [lambda: k_chunk(3, 1), lambda: k_chunk(3, 2)],
            (2, 2): [lambda: k_chunk(3, 3), lambda: q_chunk(3, 1)],
            (2, 3): [lambda: q_chunk(3, 2)],
            (3, 0): [lambda: q_chunk(3, 3)],
            (3, 1): [lambda s=s: out_proj(s) for s in range(0, 4)],
            (3, 2): [lambda s=s: out_proj(s) for s in range(4, 8)],
            (3, 3): [lambda s=s: out_proj(s) for s in range(8, 12)],
        }

        def make_mid(items, off=0):
            n = len(items)
            pos = [min((k * 8) // n + off, 7) for k in range(n)] if n else []
            idx = [0]

            def mid(kt2):
                while idx[0] < n and pos[idx[0]] <= kt2:
                    items[idx[0]]()
                    idx[0] += 1
            return mid

        for hp in range(NHP):
            for qc in range(4):
                if (hp, qc) == (0, 0):
                    attention(hp, qc, emit_mid=first_mid)
                else:
                    attention(hp, qc,
                              emit_mid=make_mid(fillers.get((hp, qc), []),
                                                off=4))
        last_pv, last_fin = carry[0]
        last_pv()
        last_fin(on_act=True)
        carry[0] = None
        for st in range(12, 16):
            out_proj(st, flush=True)
        if dump:
            nc.sync.dma_start(out=qt_d.ap(), in_=qt[:])
            nc.sync.dma_start(out=kt_d.ap(), in_=kt[:])
            nc.sync.dma_start(out=v_d.ap(), in_=v_sb[:])
            nc.sync.dma_start(out=ctx_d.ap(), in_=ctx_sb[:])

    nc.finalize()
    return nc


_NC = {}


def _get_nc(with_bias=True):
    if with_bias not in _NC:
        _NC[with_bias] = _build_kernel(with_bias)
    return _NC[with_bias]


def _f8_split(a):
    hi = a.astype(NP_F8)
    lo = (a - hi.astype(np.float32)).astype(NP_F8)
    return hi, lo


def _host_prep(hidden_states, Wq, bq, Wk, bk, Wv, bv, Wo, bo):
    """Build per-core input maps (host does layout transforms only)."""
    f32 = np.float32
    hidden_states = np.asarray(hidden_states, f32)
    Wq, Wk, Wv, Wo = (np.asarray(w, f32) for w in (Wq, Wk, Wv, Wo))
    bq, bk, bv, bo = (np.asarray(b, f32) for b in (bq, bk, bv, bo))

    # rope interleave: new row 64*blk + 2*i + t <- old row 64*blk + 32*t + i
    p = np.arange(D)
    blk, r = p // HD, p % HD
    perm = blk * HD + (r % 2) * 32 + (r // 2)

    # per-half weight slices, scaled by WSC, transposed, fp8 hi/lo split
    halves = []
    for hh in range(2):
        rows = slice(hh * EH, (hh + 1) * EH)
        wq_t = np.ascontiguousarray((Wq[perm][rows] * WSC).T)  # [D, EH]
        wk_t = np.ascontiguousarray((Wk[perm][rows] * WSC).T)
        wv_t = np.ascontiguousarray((Wv[rows] * WSC).T)
        wo_t = np.ascontiguousarray((Wo.T[rows] * WSC).reshape(NHP, 128, D))
        m = {}
        for nm, arr in (("wq", wq_t), ("wk", wk_t), ("wv", wv_t),
                        ("wo", wo_t)):
            m[nm + "h"], m[nm + "l"] = _f8_split(arr)
        m["bq"] = (bq[perm][rows] * WSC).reshape(1, EH).astype(NP_BF16)
        m["bk"] = (bk[perm][rows] * WSC).reshape(1, EH).astype(NP_BF16)
        halves.append(m)

    # rope tables (reference quirk: "c" is sin, "s" is cos), interleaved rows
    inv_freq = 1.0 / (10000.0 ** (np.arange(0, HD, 2, dtype=f32) / HD))
    ang = np.arange(S, dtype=f32)[:, None] * inv_freq[None, :]  # [S, 32]
    sin_t, cos_t = np.sin(ang), np.cos(ang)
    rows = np.arange(128)
    i_of = (rows % HD) // 2
    sign = np.where(rows % 2 == 0, -1.0, 1.0)
    cc = np.ascontiguousarray(sin_t.T[i_of, :]).astype(NP_BF16)        # [128, S]
    sg = np.ascontiguousarray(cos_t.T[i_of, :] * sign[:, None]).astype(NP_BF16)

    in_maps = []
    for c in range(NCORE):
        b_i, hh = c // 2, c % 2
        xt = np.ascontiguousarray(hidden_states[b_i].T)  # [D, S]
        xh, xl = _f8_split(xt)
        m = {"xh": xh, "xl": xl, "cc": cc, "sg": sg}
        m.update(halves[hh])
        in_maps.append(m)
    out_const = (Wo @ bv + bo).astype(f32)
    return in_maps, out_const


def kernel(hidden_states, Wq, bq, Wk, bk, Wv, bv, Wo, bo, _trace=False):
    in_maps, out_const = _host_prep(hidden_states, Wq, bq, Wk, bk, Wv, bv,
                                    Wo, bo)
    with_bias = bool(np.any(np.asarray(bq)) or np.any(np.asarray(bk)))
    nc = _get_nc(with_bias)
    res = run_bass_kernel_spmd(nc, in_maps, core_ids=list(range(NCORE)),
                               trace=_trace)
    out = np.empty((B, S, D), np.float32)
    for b_i in range(B):
        out[b_i] = np.asarray(res.results[2 * b_i]["out"], np.float32)
        out[b_i] += np.asarray(res.results[2 * b_i + 1]["out"], np.float32)
    out += out_const[None, None, :]
    if _trace:
        return out, res
    return out
